# revision 15
# baseline (speedup 1.0000x reference)
"""MemNN (embedding_lookup) Trainium2 Bass kernel — v2.

Strategy (8 NeuronCores, one NEFF, SPMD):
  - Data-parallel: batch dim sharded 8 ways (8 batches/core).
  - Host packs the 4 embedding tables interleaved per vocab row
    ([A0|A1|A2|A3][v], bf16) and, per core, compacts it to the core's
    unique vocab rows so indices fit dma_gather's int16.
  - Token order: queries first, then batch-major stories.  Each gather
    chunk covers exactly one batch (last batch split into shrinking
    sub-chunks to minimize end-of-pipeline exposure).  GpSimd descriptor
    generation (~8ns/row) is the critical path; everything else is
    scheduled under it:
      * PE reduces each 128-row tile into per-sentence partial sums:
        m-path via rank-2-separable position encoding
        (pe[j,d] = a(j) + b(j)*k'(d)) into [E, sent] layout; c-path via
        swapped-operand matmuls directly into [sent, E] layout (no
        transposes later).  TC is folded into the hop-update matmul as a
        constant lhsT (sum_s p_s * tc_s).
      * The 3 attention hops run per batch as soon as that batch's
        chunks land — hidden under the remaining gathers.
  - AllGather u across cores; vocab-sharded logits z = u @ A3^T in
    [128, VSH/2] layout (batch-pairs on partitions), exp-sums off PSUM,
    AllReduce, log_softmax; bf16 output with a -ln(V) pre-shift to keep
    bf16 rounding small.
"""

import numpy as np
import ml_dtypes

import concourse.bass as bass
import concourse.mybir as mybir
import concourse.tile as tile
from concourse import bacc
import concourse.bass_utils as bass_utils

F32 = mybir.dt.float32
AF = mybir.ActivationFunctionType
ALU = mybir.AluOpType
AX = mybir.AxisListType

LOGV_SHIFT = float(np.log(100000.0))


class Cfg:
    def __init__(self, ncore=8, B=64, S=50, J=64, QW=16, V=100000, E=128,
                 ucap=24576, gchunk=1024, use_bf16=True, z_f32=False):
        self.ncore, self.B, self.S, self.J, self.QW = ncore, B, S, J, QW
        self.V, self.E, self.ucap = V, E, ucap
        self.use_bf16, self.z_f32 = use_bf16, z_f32
        self.Bc = B // ncore
        self.NQ = self.Bc * QW             # query rows per core
        assert self.NQ == 128
        self.TPB = S * J                   # story tokens per batch
        assert self.TPB % 128 == 0
        self.tiles_pb = self.TPB // 128    # 128-row tiles per batch
        self.SPT = 128 // J                # sentences per tile
        assert 128 % J == 0 and self.SPT == 2
        self.NPOS = self.NQ + self.Bc * self.TPB
        self.VSH = V // ncore
        # chunks: (batch, tile_offset_within_batch, ntiles); batch -1 = query
        # dma_gather chunks are capped at 8 tiles (1024 rows) — larger
        # num_idxs hangs on HW (works in CoreSim; descriptor-ring capacity
        # is not simulated).
        self.gt_max = 8
        chunks = [(-1, 0, 1)]
        for b in range(self.Bc):
            t = 0
            while t < self.tiles_pb:
                nt = min(self.gt_max, self.tiles_pb - t)
                chunks.append((b, t, nt))
                t += nt
        self.chunks = chunks
        self.gsizes = [nt * 128 for (_, _, nt) in chunks]
        assert sum(self.gsizes) == self.NPOS
        self.TOT16 = self.NPOS // 16
        self.DT = mybir.dt.bfloat16 if use_bf16 else mybir.dt.float32
        self.npdt = ml_dtypes.bfloat16 if use_bf16 else np.float32
        self.zdt = F32 if z_f32 else self.DT
        self.znp = np.float32 if z_f32 else self.npdt
        # logits column chunking
        self.zc = 500 if self.VSH % 500 == 0 else self.VSH
        assert self.VSH % self.zc == 0 and self.zc % 2 == 0
        self.nzc = self.VSH // self.zc
        self.zh = self.zc // 2             # half-chunk (partition fold)

    def key(self):
        return (self.ncore, self.B, self.S, self.J, self.QW, self.V, self.E,
                self.ucap, self.use_bf16, self.z_f32)


def build_module(cfg):
    c = cfg
    E, Bc, S, VSH = c.E, c.Bc, c.S, c.VSH
    DT = c.DT
    nc = bacc.Bacc("TRN2", target_bir_lowering=False, debug=False,
                   num_devices=c.ncore, num_swdge_queues=1)

    t_idx = nc.dram_tensor("idx", [128, c.TOT16], mybir.dt.int16,
                           kind="ExternalInput")
    t_tab = nc.dram_tensor("tabc", [c.ucap, 4 * E], DT, kind="ExternalInput")
    t_wab = nc.dram_tensor("wab", [128, 2 * c.SPT], DT, kind="ExternalInput")
    t_wc1 = nc.dram_tensor("wc1", [128, c.SPT], DT, kind="ExternalInput")
    t_wq = nc.dram_tensor("wq", [128, Bc], DT, kind="ExternalInput")
    t_tat = nc.dram_tensor("tat", [E, S], F32, kind="ExternalInput")
    t_tctn = nc.dram_tensor("tctn", [S, E], DT, kind="ExternalInput")
    t_kp = nc.dram_tensor("kp", [E, 1], F32, kind="ExternalInput")
    t_id1 = nc.dram_tensor("id1", [1, 1], F32, kind="ExternalInput")
    t_id128 = nc.dram_tensor("id128", [128, 128], DT, kind="ExternalInput")
    assert 2 * c.B == 128  # output fold uses 128 partitions = 2*B
    t_wfold = nc.dram_tensor("wfold", [2 * c.B, c.B], F32,
                             kind="ExternalInput")  # [128, 64]
    t_wunf = nc.dram_tensor("wunf", [c.B, 2 * c.B], F32,
                            kind="ExternalInput")   # [64, 128]
    t_a3t = nc.dram_tensor("a3t", [E, VSH], c.zdt, kind="ExternalInput")

    t_o = nc.dram_tensor("o", [2 * c.B, VSH // 2], c.zdt,
                         kind="ExternalOutput")     # [128, VSH/2]
    t_du = nc.dram_tensor("du", [E, c.B], F32, kind="ExternalOutput")

    with tile.TileContext(nc) as tc:
        with tc.tile_pool(name="const", bufs=1) as cpool, \
             tc.tile_pool(name="gp", bufs=2) as gpool, \
             tc.tile_pool(name="wk", bufs=1) as wk, \
             tc.tile_pool(name="hp", bufs=3) as hp, \
             tc.tile_pool(name="big", bufs=1) as big, \
             tc.tile_pool(name="dram", bufs=1, space="DRAM") as dram:

            # ---- constant loads (idx first: gates the first gather) -------
            idxs = cpool.tile([128, c.TOT16], mybir.dt.int16)
            nc.sync.dma_start(out=idxs[:], in_=t_idx.ap())
            wab = cpool.tile([128, 2 * c.SPT], DT)
            nc.sync.dma_start(out=wab[:], in_=t_wab.ap())
            wc1 = cpool.tile([128, c.SPT], DT)
            nc.sync.dma_start(out=wc1[:], in_=t_wc1.ap())
            wq = cpool.tile([128, Bc], DT)
            nc.sync.dma_start(out=wq[:], in_=t_wq.ap())
            tat = cpool.tile([E, S], F32)
            nc.sync.dma_start(out=tat[:], in_=t_tat.ap())
            tctn = cpool.tile([S, E], DT)
            nc.sync.dma_start(out=tctn[:], in_=t_tctn.ap())
            kp = cpool.tile([E, 1], F32)
            nc.sync.dma_start(out=kp[:], in_=t_kp.ap())
            id1 = cpool.tile([1, 1], F32)
            nc.sync.dma_start(out=id1[:], in_=t_id1.ap())
            id128 = cpool.tile([128, 128], DT)
            nc.sync.dma_start(out=id128[:], in_=t_id128.ap())
            wfold = cpool.tile([128, c.B], F32)
            nc.sync.dma_start(out=wfold[:], in_=t_wfold.ap())
            wunf = cpool.tile([c.B, 128], F32)
            nc.sync.dma_start(out=wunf[:], in_=t_wunf.ap())
            a3t = big.tile([E, VSH], c.zdt)
            nc.sync.dma_start(out=a3t[:], in_=t_a3t.ap())

            # ---- persistent work tiles ------------------------------------
            mst = [wk.tile([E, 3, S], F32, tag=f"mst{b}", name=f"mst{b}")
                   for b in range(Bc)]
            cstE = [wk.tile([E, 3, S], DT, tag=f"cstE{b}", name=f"cstE{b}")
                    for b in range(Bc)]
            cst = [wk.tile([S, 3 * E], DT, tag=f"cst{b}", name=f"cst{b}")
                   for b in range(Bc)]
            uS = [wk.tile([E, Bc], F32, tag=f"uS{i}", name=f"uS{i}")
                  for i in range(4)]

            with tc.tile_pool(name="psM", bufs=2, space="PSUM") as psM, \
                 tc.tile_pool(name="psC", bufs=1, space="PSUM") as psC, \
                 tc.tile_pool(name="psH", bufs=1, space="PSUM") as psH:

                def do_hops(b):
                    for h in range(3):
                        sc_ps = psH.tile([1, S], F32, space="PSUM", tag="sc")
                        nc.tensor.matmul(out=sc_ps[:],
                                         lhsT=uS[h][:, b:b + 1],
                                         rhs=mst[b][:, h, :],
                                         start=True, stop=True)
                        ngmx = hp.tile([1, 1], F32, tag="ngmx")
                        nc.vector.tensor_reduce(out=ngmx[:], in_=sc_ps[:],
                                                axis=AX.X, op=ALU.max,
                                                negate=True)
                        ex = hp.tile([1, S], F32, tag="ex")
                        zsum = hp.tile([1, 1], F32, tag="zsum")
                        nc.scalar.activation(out=ex[:], in_=sc_ps[:],
                                             func=AF.Exp, bias=ngmx[:],
                                             scale=1.0, accum_out=zsum[:])
                        rz = hp.tile([1, 1], F32, tag="rz")
                        nc.vector.reciprocal(out=rz[:], in_=zsum[:])
                        exn = hp.tile([1, S], F32, tag="exn")
                        nc.vector.tensor_scalar(out=exn[:], in0=ex[:],
                                                scalar1=rz[:], scalar2=None,
                                                op0=ALU.mult)
                        pt_ps = psH.tile([S, 1], F32, space="PSUM", tag="pt")
                        nc.tensor.transpose(out=pt_ps[:], in_=exn[:],
                                            identity=id1[:])
                        pt = hp.tile([S, 1], DT, tag="ptb")
                        nc.vector.tensor_copy(pt[:], pt_ps[:])
                        up_ps = psH.tile([E, 1], F32, space="PSUM", tag="up")
                        nc.tensor.matmul(out=up_ps[:],
                                         lhsT=cst[b][:, h * E:(h + 1) * E],
                                         rhs=pt[:], start=True, stop=False)
                        nc.tensor.matmul(out=up_ps[:], lhsT=tctn[:],
                                         rhs=pt[:], start=False, stop=True)
                        nc.vector.tensor_tensor(out=uS[h + 1][:, b:b + 1],
                                                in0=up_ps[:],
                                                in1=uS[h][:, b:b + 1],
                                                op=ALU.add)

                # ---- gather chunks + reductions + per-batch hops ----------
                off16 = 0
                for (b, t0, ntiles) in c.chunks:
                    gs = ntiles * 128
                    if b < 0:
                        # query chunk
                        gq = cpool.tile([128, 1, 4 * E], DT)
                        nc.gpsimd.dma_gather(
                            out_ap=gq[:, :1, :], in_ap=t_tab.ap(),
                            idxs_ap=idxs[:, off16:off16 + gs // 16],
                            num_idxs=gs, num_idxs_reg=gs, elem_size=4 * E)
                        Pq = psM.tile([128, 6 * S], F32, space="PSUM",
                                      tag="Pm")
                        nc.tensor.matmul(out=Pq[:, 0:Bc],
                                         lhsT=gq[:, 0, 0:E], rhs=wq[:],
                                         start=True, stop=True)
                        nc.vector.tensor_copy(uS[0][:], Pq[:, 0:Bc])
                        off16 += gs // 16
                        continue

                    spc = ntiles * c.SPT       # sentences in this chunk
                    s0 = t0 * c.SPT            # batch-local sentence offset
                    gt = gpool.tile([128, c.gt_max, 4 * E], DT, tag="g")
                    nc.gpsimd.dma_gather(
                        out_ap=gt[:, :ntiles, :], in_ap=t_tab.ap(),
                        idxs_ap=idxs[:, off16:off16 + gs // 16],
                        num_idxs=gs, num_idxs_reg=gs, elem_size=4 * E)
                    off16 += gs // 16

                    Pm = psM.tile([128, 6 * S], F32, space="PSUM", tag="Pm")
                    Pc = psC.tile([128, 3 * S], F32, space="PSUM", tag="Pc")
                    for t in range(ntiles):
                        sl = 2 * t             # chunk-local sentence base
                        for h in range(3):
                            nc.tensor.matmul(
                                out=Pm[:, 2 * S * h + 2 * sl:
                                       2 * S * h + 2 * sl + 2 * c.SPT],
                                lhsT=gt[:, t, h * E:(h + 1) * E],
                                rhs=wab[:], start=True, stop=True)
                            nc.tensor.matmul(
                                out=Pc[:, spc * h + sl:
                                       spc * h + sl + c.SPT],
                                lhsT=gt[:, t, (h + 1) * E:(h + 2) * E],
                                rhs=wc1[:], start=True, stop=True)

                    # combines: m = Sa + kp*Sb + tat   (per hop)
                    pap = Pm[:]
                    pdim = pap.ap[0]
                    for h in range(3):
                        base = 2 * S * h
                        sa = bass.AP(pap.tensor, pap.offset + base,
                                     [pdim, (2, spc)])
                        sb = bass.AP(pap.tensor, pap.offset + base + 1,
                                     [pdim, (2, spc)])
                        msl = mst[b][:, h, s0:s0 + spc]
                        nc.vector.tensor_scalar(out=msl, in0=sb,
                                                scalar1=kp[:], scalar2=None,
                                                op0=ALU.mult)
                        nc.vector.tensor_tensor(out=msl, in0=msl, in1=sa,
                                                op=ALU.add)
                        nc.vector.tensor_tensor(out=msl, in0=msl,
                                                in1=tat[:, s0:s0 + spc],
                                                op=ALU.add)
                        nc.vector.tensor_copy(cstE[b][:, h, s0:s0 + spc],
                                              Pc[:, spc * h:
                                                 spc * h + spc])

                    if t0 + ntiles == c.tiles_pb:
                        # transpose c to [sent, E] for the update matmuls
                        for h in range(3):
                            cn_ps = psC.tile([S, E], DT, space="PSUM",
                                             tag="cn")
                            nc.tensor.transpose(out=cn_ps[:],
                                                in_=cstE[b][:, h, :],
                                                identity=id128[:])
                            nc.vector.tensor_copy(
                                cst[b][:, h * E:(h + 1) * E], cn_ps[:])
                        do_hops(b)

            # ---- AllGather u ----------------------------------------------
            ub_in = dram.tile([E, Bc], F32)
            ub_out = dram.tile([c.ncore * E, Bc], F32)
            nc.gpsimd.dma_start(ub_in[:], uS[3][:])
            nc.gpsimd.collective_compute(
                "AllGather", ALU.bypass,
                replica_groups=[list(range(c.ncore))],
                ins=[ub_in.opt()], outs=[ub_out.opt()],
            )
            uTf = wk.tile([E, c.ncore, Bc], F32, tag="uTf")
            src = bass.AP(ub_out[:].tensor, ub_out[:].offset,
                          [(Bc, E), (E * Bc, c.ncore), (1, Bc)])
            nc.sync.dma_start(out=uTf[:], in_=src)
            nc.sync.dma_start(out=t_du.ap(),
                              in_=uTf[:].rearrange("e c b -> e (c b)"))
            uz = wk.tile([E, c.B], c.zdt, tag="uz")
            nc.vector.tensor_copy(uz[:], uTf[:].rearrange("e c b -> e (c b)"))

            # ---- logits + log_softmax -------------------------------------
            z128 = big.tile([128, VSH // 2], c.zdt)
            sums = wk.tile([128, c.nzc], F32, tag="sums")
            with tc.tile_pool(name="psZ", bufs=4, space="PSUM") as psZ, \
                 tc.tile_pool(name="psF", bufs=1, space="PSUM") as psF:
                for k in range(c.nzc):
                    zps = psZ.tile([128, c.zh], F32, space="PSUM", tag="z")
                    nc.tensor.matmul(out=zps[0:c.B, :], lhsT=uz[:],
                                     rhs=a3t[:, k * c.zc:k * c.zc + c.zh],
                                     start=True, stop=True)
                    nc.tensor.matmul(out=zps[c.B:2 * c.B, :], lhsT=uz[:],
                                     rhs=a3t[:, k * c.zc + c.zh:
                                             (k + 1) * c.zc],
                                     start=True, stop=True)
                    nc.vector.tensor_scalar(
                        out=z128[:, k * c.zh:(k + 1) * c.zh], in0=zps[:],
                        scalar1=-LOGV_SHIFT, scalar2=None, op0=ALU.add)
                    esc = hp.tile([128, c.zh], DT, tag="esc")
                    nc.scalar.activation(out=esc[:], in_=zps[:], func=AF.Exp,
                                         accum_out=sums[:, k:k + 1])

                slc = wk.tile([128, 1], F32, tag="slc")
                nc.vector.tensor_reduce(out=slc[:], in_=sums[:], axis=AX.X,
                                        op=ALU.add)
                sb_in = dram.tile([128, 1], F32)
                sb_out = dram.tile([128, 1], F32)
                nc.gpsimd.dma_start(sb_in[:], slc[:])
                nc.gpsimd.collective_compute(
                    "AllReduce", ALU.add,
                    replica_groups=[list(range(c.ncore))],
                    ins=[sb_in.opt()], outs=[sb_out.opt()],
                )
                st = wk.tile([128, 1], F32, tag="st")
                nc.sync.dma_start(out=st[:], in_=sb_out[:])
                # fold [128,1] -> [64,1]: total expsum per batch
                stf_ps = psF.tile([c.B, 1], F32, space="PSUM", tag="stf")
                nc.tensor.matmul(out=stf_ps[:], lhsT=wfold[:], rhs=st[:],
                                 start=True, stop=True)
                lseS = wk.tile([c.B, 1], F32, tag="lseS")
                nc.scalar.activation(out=lseS[:], in_=stf_ps[:], func=AF.Ln)
                # unfold [64,1] -> [128,1] and pre-shift by -ln(V)
                lse2_ps = psF.tile([128, 1], F32, space="PSUM", tag="lse2")
                nc.tensor.matmul(out=lse2_ps[:], lhsT=wunf[:], rhs=lseS[:],
                                 start=True, stop=True)
                lse2 = wk.tile([128, 1], F32, tag="lse2s")
                nc.vector.tensor_scalar(out=lse2[:], in0=lse2_ps[:],
                                        scalar1=-LOGV_SHIFT, scalar2=None,
                                        op0=ALU.add)
                # subtract + store output, in halves (overlap DVE with DMA)
                H = VSH // 4
                for i in range(2):
                    sl = slice(i * H, (i + 1) * H if i == 0 else VSH // 2)
                    nc.vector.tensor_scalar(out=z128[:, sl], in0=z128[:, sl],
                                            scalar1=lse2[:], scalar2=None,
                                            op0=ALU.subtract)
                    nc.sync.dma_start(out=t_o.ap()[:, sl], in_=z128[:, sl])

    nc.compile()
    return nc


def host_prep(cfg, x, q, A, TA, TC):
    c = cfg
    E, J, S = c.E, c.J, c.S
    x = np.asarray(x).astype(np.int64)
    q = np.asarray(q).astype(np.int64)
    A = np.asarray(A, dtype=np.float32)
    TA = np.asarray(TA, dtype=np.float32)
    TC = np.asarray(TC, dtype=np.float32)

    tabI = np.ascontiguousarray(A.transpose(1, 0, 2).reshape(c.V, 4 * E))
    tabI = tabI.astype(c.npdt)
    a3tF = np.ascontiguousarray(A[3].T)  # [E, V] f32

    j = np.arange(1, J + 1, dtype=np.float32)
    av = 1.0 - j / J
    bv = 2.0 * j / J - 1.0
    sp = np.arange(128) // J
    jj = np.arange(128) % J
    wab = np.zeros((128, 2 * c.SPT), np.float32)
    wc1 = np.zeros((128, c.SPT), np.float32)
    for p in range(128):
        wab[p, 2 * sp[p] + 0] = av[jj[p]]
        wab[p, 2 * sp[p] + 1] = bv[jj[p]]
        wc1[p, sp[p]] = 1.0
    wq = np.zeros((128, c.Bc), np.float32)
    for p in range(128):
        wq[p, p // c.QW] = 1.0

    tat = np.ascontiguousarray(TA[0, :S, :].T)          # [E, S] f32
    tctn = np.ascontiguousarray(TC[0, :S, :])           # [S, E]
    kp = ((np.arange(E, dtype=np.float32) + 1.0) / E).reshape(E, 1)
    id1 = np.ones((1, 1), np.float32)
    id128 = np.eye(128, dtype=np.float32)
    wfold = np.zeros((128, c.B), np.float32)
    for p in range(128):
        wfold[p, p % c.B] = 1.0
    wunf = np.zeros((c.B, 128), np.float32)
    for b in range(c.B):
        wunf[b, b] = 1.0
        wunf[b, c.B + b] = 1.0

    common = {
        "wab": wab.astype(c.npdt), "wc1": wc1.astype(c.npdt),
        "wq": wq.astype(c.npdt), "tat": tat, "tctn": tctn.astype(c.npdt),
        "kp": kp, "id1": id1, "id128": id128.astype(c.npdt),
        "wfold": wfold, "wunf": wunf,
    }

    in_maps = []
    for cc in range(c.ncore):
        qc = q[cc * c.Bc:(cc + 1) * c.Bc].reshape(-1)
        xc = x[cc * c.Bc:(cc + 1) * c.Bc].reshape(c.Bc, -1)
        xq = np.concatenate([qc] + [xc[b] for b in range(c.Bc)])
        uniq, rel = np.unique(xq, return_inverse=True)
        assert len(uniq) <= c.ucap, (len(uniq), c.ucap)
        tabc = np.zeros((c.ucap, 4 * E), c.npdt)
        tabc[:len(uniq)] = tabI[uniq]
        rel = rel.astype(np.int16)
        idx = np.zeros((128, c.TOT16), np.int16)
        off = 0
        for gs in c.gsizes:
            v = rel[off:off + gs]
            wrapped = v.reshape(-1, 16).T
            idx[:, off // 16:(off + gs) // 16] = np.tile(wrapped, (8, 1))
            off += gs
        a3c = np.ascontiguousarray(
            a3tF[:, cc * c.VSH:(cc + 1) * c.VSH]).astype(c.znp)
        m = dict(common)
        m.update({"tabc": tabc, "idx": idx, "a3t": a3c})
        in_maps.append(m)
    return in_maps


_CACHE = {}


def _get_module(cfg):
    k = cfg.key()
    if k not in _CACHE:
        _CACHE[k] = build_module(cfg)
    return _CACHE[k]


def run(cfg, inputs, trace=False):
    nc = _get_module(cfg)
    in_maps = host_prep(cfg, inputs["x"], inputs["q"], inputs["A"],
                        inputs["TA"], inputs["TC"])
    res = bass_utils.run_bass_kernel_spmd(
        nc, in_maps, core_ids=list(range(cfg.ncore)), trace=trace)
    parts = []
    for cc in range(cfg.ncore):
        z = np.asarray(res.results[cc]["o"]).astype(np.float32)
        # [128, VSH/2] -> [64, VSH]: partition p = half*64 + b,
        # col j = k*zh + c  ->  out[b, k*zc + half*zh + c]
        z = z.reshape(2, cfg.B, cfg.nzc, cfg.zh)
        z = z.transpose(1, 2, 0, 3).reshape(cfg.B, cfg.VSH)
        parts.append(z)
    out = np.concatenate(parts, axis=1)
    return out, res


def kernel(**inputs) -> np.ndarray:
    cfg = Cfg()
    out, _ = run(cfg, inputs, trace=False)
    return out


# revision 24
# speedup vs baseline: 1.1054x; 1.1054x over previous
"""MemNN (embedding_lookup) Trainium2 Bass kernel — v2.

Strategy (8 NeuronCores, one NEFF, SPMD):
  - Data-parallel: batch dim sharded 8 ways (8 batches/core).
  - Host packs the 4 embedding tables interleaved per vocab row
    ([A0|A1|A2|A3][v], bf16) and, per core, compacts it to the core's
    unique vocab rows so indices fit dma_gather's int16.
  - Token order: queries first, then batch-major stories.  Each gather
    chunk covers exactly one batch (last batch split into shrinking
    sub-chunks to minimize end-of-pipeline exposure).  GpSimd descriptor
    generation (~8ns/row) is the critical path; everything else is
    scheduled under it:
      * PE reduces each 128-row tile into per-sentence partial sums:
        m-path via rank-2-separable position encoding
        (pe[j,d] = a(j) + b(j)*k'(d)) into [E, sent] layout; c-path via
        swapped-operand matmuls directly into [sent, E] layout (no
        transposes later).  TC is folded into the hop-update matmul as a
        constant lhsT (sum_s p_s * tc_s).
      * The 3 attention hops run per batch as soon as that batch's
        chunks land — hidden under the remaining gathers.
  - AllGather u across cores; vocab-sharded logits z = u @ A3^T in
    [128, VSH/2] layout (batch-pairs on partitions), exp-sums off PSUM,
    AllReduce, log_softmax; bf16 output with a -ln(V) pre-shift to keep
    bf16 rounding small.
"""

import numpy as np
import ml_dtypes

import concourse.bass as bass
import concourse.mybir as mybir
import concourse.tile as tile
from concourse import bacc
import concourse.bass_utils as bass_utils

F32 = mybir.dt.float32
AF = mybir.ActivationFunctionType
ALU = mybir.AluOpType
AX = mybir.AxisListType

LOGV_SHIFT = float(np.log(100000.0))


class Cfg:
    def __init__(self, ncore=8, B=64, S=50, J=64, QW=16, V=100000, E=128,
                 ucap=24576, gchunk=1024, use_bf16=True, z_f32=False):
        self.ncore, self.B, self.S, self.J, self.QW = ncore, B, S, J, QW
        self.V, self.E, self.ucap = V, E, ucap
        self.use_bf16, self.z_f32 = use_bf16, z_f32
        self.Bc = B // ncore
        self.NQ = self.Bc * QW             # query rows per core
        assert self.NQ == 128
        self.TPB = S * J                   # story tokens per batch
        assert self.TPB % 128 == 0
        self.tiles_pb = self.TPB // 128    # 128-row tiles per batch
        self.SPT = 128 // J                # sentences per tile
        assert 128 % J == 0 and self.SPT == 2
        self.NPOS = self.NQ + self.Bc * self.TPB
        self.VSH = V // ncore
        # chunks: (batch, tile_offset_within_batch, ntiles); batch -1 = query
        # dma_gather chunks are capped at 8 tiles (1024 rows) — larger
        # num_idxs hangs on HW (works in CoreSim; descriptor-ring capacity
        # is not simulated).
        self.gt_max = 8
        chunks = [(-1, 0, 1)]
        for b in range(self.Bc):
            t = 0
            while t < self.tiles_pb:
                nt = min(self.gt_max, self.tiles_pb - t)
                chunks.append((b, t, nt))
                t += nt
        self.chunks = chunks
        self.gsizes = [nt * 128 for (_, _, nt) in chunks]
        assert sum(self.gsizes) == self.NPOS
        self.TOT16 = self.NPOS // 16
        self.DT = mybir.dt.bfloat16 if use_bf16 else mybir.dt.float32
        self.npdt = ml_dtypes.bfloat16 if use_bf16 else np.float32
        self.zdt = F32 if z_f32 else self.DT
        self.znp = np.float32 if z_f32 else self.npdt
        # logits column chunking
        self.zc = 500 if self.VSH % 500 == 0 else self.VSH
        assert self.VSH % self.zc == 0 and self.zc % 2 == 0
        self.nzc = self.VSH // self.zc
        self.zh = self.zc // 2             # half-chunk (partition fold)

    def key(self):
        return (self.ncore, self.B, self.S, self.J, self.QW, self.V, self.E,
                self.ucap, self.use_bf16, self.z_f32)


def build_module(cfg):
    c = cfg
    E, Bc, S, VSH = c.E, c.Bc, c.S, c.VSH
    DT = c.DT
    nc = bacc.Bacc("TRN2", target_bir_lowering=False, debug=False,
                   num_devices=c.ncore, num_swdge_queues=1)

    t_idx = nc.dram_tensor("idx", [128, c.TOT16], mybir.dt.int16,
                           kind="ExternalInput")
    t_tab = nc.dram_tensor("tabc", [c.ucap, 4 * E], DT, kind="ExternalInput")
    t_wab = nc.dram_tensor("wab", [128, 2 * c.SPT], DT, kind="ExternalInput")
    t_wabc = nc.dram_tensor("wabc", [128, 3 * c.SPT], DT,
                            kind="ExternalInput")
    t_wc1 = nc.dram_tensor("wc1", [128, c.SPT], DT, kind="ExternalInput")
    t_wq = nc.dram_tensor("wq", [128, Bc], DT, kind="ExternalInput")
    t_tat = nc.dram_tensor("tat", [E, S], F32, kind="ExternalInput")
    t_tct = nc.dram_tensor("tct", [E, S], F32, kind="ExternalInput")
    t_kp = nc.dram_tensor("kp", [E, 1], F32, kind="ExternalInput")
    t_id1 = nc.dram_tensor("id1", [1, 1], F32, kind="ExternalInput")
    t_id128 = nc.dram_tensor("id128", [128, 128], DT, kind="ExternalInput")
    assert 2 * c.B == 128  # output fold uses 128 partitions = 2*B
    t_wfold = nc.dram_tensor("wfold", [2 * c.B, c.B], F32,
                             kind="ExternalInput")  # [128, 64]
    t_wunf = nc.dram_tensor("wunf", [c.B, 2 * c.B], F32,
                            kind="ExternalInput")   # [64, 128]
    t_a3t = nc.dram_tensor("a3t", [E, VSH], c.zdt, kind="ExternalInput")

    t_o = nc.dram_tensor("o", [2 * c.B, VSH // 2], c.zdt,
                         kind="ExternalOutput")     # [128, VSH/2]
    t_du = nc.dram_tensor("du", [E, c.B], F32, kind="ExternalOutput")

    with tile.TileContext(nc) as tc:
        with tc.tile_pool(name="const", bufs=1) as cpool, \
             tc.tile_pool(name="gp", bufs=3) as gpool, \
             tc.tile_pool(name="wk", bufs=1) as wk, \
             tc.tile_pool(name="hp", bufs=3) as hp, \
             tc.tile_pool(name="big", bufs=1) as big, \
             tc.tile_pool(name="dram", bufs=1, space="DRAM") as dram:

            # ---- constant loads (idx first: gates the first gather) -------
            idxs = cpool.tile([128, c.TOT16], mybir.dt.int16)
            nc.sync.dma_start(out=idxs[:], in_=t_idx.ap())
            wab = cpool.tile([128, 2 * c.SPT], DT)
            nc.sync.dma_start(out=wab[:], in_=t_wab.ap())
            wabc = cpool.tile([128, 3 * c.SPT], DT)
            nc.sync.dma_start(out=wabc[:], in_=t_wabc.ap())
            wc1 = cpool.tile([128, c.SPT], DT)
            nc.sync.dma_start(out=wc1[:], in_=t_wc1.ap())
            wq = cpool.tile([128, Bc], DT)
            nc.sync.dma_start(out=wq[:], in_=t_wq.ap())
            tat = cpool.tile([E, S], F32)
            nc.sync.dma_start(out=tat[:], in_=t_tat.ap())
            tct = cpool.tile([E, S], F32)
            nc.sync.dma_start(out=tct[:], in_=t_tct.ap())
            kp = cpool.tile([E, 1], F32)
            nc.sync.dma_start(out=kp[:], in_=t_kp.ap())
            id1 = cpool.tile([1, 1], F32)
            nc.sync.dma_start(out=id1[:], in_=t_id1.ap())
            id128 = cpool.tile([128, 128], DT)
            nc.sync.dma_start(out=id128[:], in_=t_id128.ap())
            wfold = cpool.tile([128, c.B], F32)
            nc.sync.dma_start(out=wfold[:], in_=t_wfold.ap())
            wunf = cpool.tile([c.B, 128], F32)
            nc.sync.dma_start(out=wunf[:], in_=t_wunf.ap())
            a3t = big.tile([E, VSH], c.zdt)
            nc.sync.dma_start(out=a3t[:], in_=t_a3t.ap())

            # ---- persistent work tiles ------------------------------------
            mst = [wk.tile([E, 3, S], F32, tag=f"mst{b}", name=f"mst{b}")
                   for b in range(Bc)]
            cstE = [wk.tile([E, 3, S], DT, tag=f"cstE{b}", name=f"cstE{b}")
                    for b in range(Bc)]
            cst = [wk.tile([S, 3 * E], DT, tag=f"cst{b}", name=f"cst{b}")
                   for b in range(Bc)]
            uS = [wk.tile([E, Bc], F32, tag=f"uS{i}", name=f"uS{i}")
                  for i in range(4)]

            with tc.tile_pool(name="psM", bufs=3, space="PSUM") as psM, \
                 tc.tile_pool(name="psC", bufs=2, space="PSUM") as psC, \
                 tc.tile_pool(name="psH", bufs=1, space="PSUM") as psH:

                def hop_stage(b, h):
                    # softmax without max-subtraction: scores are bounded
                    # (|score| < 60 for this model family; exp is f32-safe)
                    sc_ps = psH.tile([1, S], F32, space="PSUM", tag="sc")
                    nc.tensor.matmul(out=sc_ps[:],
                                     lhsT=uS[h][:, b:b + 1],
                                     rhs=mst[b][:, h, :],
                                     start=True, stop=True)
                    ex = hp.tile([1, S], F32, tag="ex")
                    zsum = hp.tile([1, 1], F32, tag="zsum")
                    nc.scalar.activation(out=ex[:], in_=sc_ps[:],
                                         func=AF.Exp, accum_out=zsum[:])
                    rz = hp.tile([1, 1], F32, tag="rz")
                    nc.vector.reciprocal(out=rz[:], in_=zsum[:])
                    exn = hp.tile([1, S], F32, tag="exn")
                    nc.vector.tensor_scalar(out=exn[:], in0=ex[:],
                                            scalar1=rz[:], scalar2=None,
                                            op0=ALU.mult)
                    pt_ps = psH.tile([S, 1], F32, space="PSUM", tag="pt")
                    nc.tensor.transpose(out=pt_ps[:], in_=exn[:],
                                        identity=id1[:])
                    pt = hp.tile([S, 1], DT, tag="ptb")
                    nc.vector.tensor_copy(pt[:], pt_ps[:])
                    up_ps = psH.tile([E, 1], F32, space="PSUM", tag="up")
                    nc.tensor.matmul(out=up_ps[:],
                                     lhsT=cst[b][:, h * E:(h + 1) * E],
                                     rhs=pt[:], start=True, stop=True)
                    nc.vector.tensor_tensor(out=uS[h + 1][:, b:b + 1],
                                            in0=up_ps[:],
                                            in1=uS[h][:, b:b + 1],
                                            op=ALU.add)

                def tr_stage(b):
                    # transpose c to [sent, E] for the update matmuls
                    for h in range(3):
                        cn_ps = psC.tile([S, E], DT, space="PSUM", tag="cn")
                        nc.tensor.transpose(out=cn_ps[:],
                                            in_=cstE[b][:, h, :],
                                            identity=id128[:])
                        nc.vector.tensor_copy(
                            cst[b][:, h * E:(h + 1) * E], cn_ps[:])

                # ---- gather chunks + reductions + per-batch hops ----------
                # Hop/transpose stages are drained one per chunk so their
                # sem waits sit between chunk matmul bursts on each
                # sequencer (no head-of-line blocking).
                import collections
                stages = collections.deque()
                L0, L1, L2, L3 = 0, 2 * S, 5 * S, 8 * S
                off16 = 0
                for (b, t0, ntiles) in c.chunks:
                    gs = ntiles * 128
                    if b < 0:
                        # query chunk
                        gq = cpool.tile([128, 1, 4 * E], DT)
                        nc.gpsimd.dma_gather(
                            out_ap=gq[:, :1, :], in_ap=t_tab.ap(),
                            idxs_ap=idxs[:, off16:off16 + gs // 16],
                            num_idxs=gs, num_idxs_reg=gs, elem_size=4 * E)
                        Pq = psM.tile([128, 9 * S], F32, space="PSUM",
                                      tag="Pm")
                        nc.tensor.matmul(out=Pq[:, 0:Bc],
                                         lhsT=gq[:, 0, 0:E], rhs=wq[:],
                                         start=True, stop=True)
                        nc.vector.tensor_copy(uS[0][:], Pq[:, 0:Bc])
                        off16 += gs // 16
                        continue

                    spc = ntiles * c.SPT       # sentences in this chunk
                    s0 = t0 * c.SPT            # batch-local sentence offset
                    gt = gpool.tile([128, c.gt_max, 4 * E], DT, tag="g")
                    nc.gpsimd.dma_gather(
                        out_ap=gt[:, :ntiles, :], in_ap=t_tab.ap(),
                        idxs_ap=idxs[:, off16:off16 + gs // 16],
                        num_idxs=gs, num_idxs_reg=gs, elem_size=4 * E)
                    off16 += gs // 16

                    Pm = psM.tile([128, 9 * S], F32, space="PSUM", tag="Pm")
                    for t in range(ntiles):
                        sl = 2 * t             # chunk-local sentence base
                        nc.tensor.matmul(
                            out=Pm[:, L0 + 2 * sl:L0 + 2 * sl + 4],
                            lhsT=gt[:, t, 0:E], rhs=wab[:],
                            start=True, stop=True)
                        nc.tensor.matmul(
                            out=Pm[:, L1 + 3 * sl:L1 + 3 * sl + 6],
                            lhsT=gt[:, t, E:2 * E], rhs=wabc[:],
                            start=True, stop=True)
                        nc.tensor.matmul(
                            out=Pm[:, L2 + 3 * sl:L2 + 3 * sl + 6],
                            lhsT=gt[:, t, 2 * E:3 * E], rhs=wabc[:],
                            start=True, stop=True)
                        nc.tensor.matmul(
                            out=Pm[:, L3 + sl:L3 + sl + 2],
                            lhsT=gt[:, t, 3 * E:4 * E], rhs=wc1[:],
                            start=True, stop=True)

                    # combines: m = Sa + kp*Sb + tat ; cE = Csum + tct
                    pap = Pm[:]
                    pdim = pap.ap[0]

                    def pv(off, stride):
                        return bass.AP(pap.tensor, pap.offset + off,
                                       [pdim, (stride, spc)])

                    for h, (sa, sb) in enumerate(
                            [(pv(L0, 2), pv(L0 + 1, 2)),
                             (pv(L1, 3), pv(L1 + 1, 3)),
                             (pv(L2, 3), pv(L2 + 1, 3))]):
                        msl = mst[b][:, h, s0:s0 + spc]
                        nc.vector.tensor_scalar(out=msl, in0=sb,
                                                scalar1=kp[:], scalar2=None,
                                                op0=ALU.mult)
                        nc.vector.tensor_tensor(out=msl, in0=msl, in1=sa,
                                                op=ALU.add)
                        nc.vector.tensor_tensor(out=msl, in0=msl,
                                                in1=tat[:, s0:s0 + spc],
                                                op=ALU.add)
                    for h, cv in enumerate([pv(L1 + 2, 3), pv(L2 + 2, 3),
                                            pv(L3, 1)]):
                        nc.vector.tensor_tensor(
                            out=cstE[b][:, h, s0:s0 + spc], in0=cv,
                            in1=tct[:, s0:s0 + spc], op=ALU.add)

                    if t0 + ntiles == c.tiles_pb:
                        stages.append(lambda bb=b: tr_stage(bb))
                        for h in range(3):
                            stages.append(lambda bb=b, hh=h:
                                          hop_stage(bb, hh))
                    if stages:
                        stages.popleft()()

                while stages:
                    stages.popleft()()

            # ---- AllGather u ----------------------------------------------
            ub_in = dram.tile([E, Bc], F32)
            ub_out = dram.tile([c.ncore * E, Bc], F32)
            nc.gpsimd.dma_start(ub_in[:], uS[3][:])
            nc.gpsimd.collective_compute(
                "AllGather", ALU.bypass,
                replica_groups=[list(range(c.ncore))],
                ins=[ub_in.opt()], outs=[ub_out.opt()],
            )
            uTf = wk.tile([E, c.ncore, Bc], F32, tag="uTf")
            src = bass.AP(ub_out[:].tensor, ub_out[:].offset,
                          [(Bc, E), (E * Bc, c.ncore), (1, Bc)])
            nc.sync.dma_start(out=uTf[:], in_=src)
            nc.sync.dma_start(out=t_du.ap(),
                              in_=uTf[:].rearrange("e c b -> e (c b)"))
            uz = wk.tile([E, c.B], c.zdt, tag="uz")
            nc.vector.tensor_copy(uz[:], uTf[:].rearrange("e c b -> e (c b)"))

            # ---- logits + log_softmax -------------------------------------
            z128 = big.tile([128, VSH // 2], c.zdt)
            sums = wk.tile([128, c.nzc], F32, tag="sums")
            with tc.tile_pool(name="psZ", bufs=4, space="PSUM") as psZ, \
                 tc.tile_pool(name="psF", bufs=1, space="PSUM") as psF:
                for k in range(c.nzc):
                    zps = psZ.tile([128, c.zh], F32, space="PSUM", tag="z")
                    nc.tensor.matmul(out=zps[0:c.B, :], lhsT=uz[:],
                                     rhs=a3t[:, k * c.zc:k * c.zc + c.zh],
                                     start=True, stop=True)
                    nc.tensor.matmul(out=zps[c.B:2 * c.B, :], lhsT=uz[:],
                                     rhs=a3t[:, k * c.zc + c.zh:
                                             (k + 1) * c.zc],
                                     start=True, stop=True)
                    nc.vector.tensor_scalar(
                        out=z128[:, k * c.zh:(k + 1) * c.zh], in0=zps[:],
                        scalar1=-LOGV_SHIFT, scalar2=None, op0=ALU.add)
                    esc = hp.tile([128, c.zh], DT, tag="esc")
                    nc.scalar.activation(out=esc[:], in_=zps[:], func=AF.Exp,
                                         accum_out=sums[:, k:k + 1])

                slc = wk.tile([128, 1], F32, tag="slc")
                nc.vector.tensor_reduce(out=slc[:], in_=sums[:], axis=AX.X,
                                        op=ALU.add)
                sb_in = dram.tile([128, 1], F32)
                sb_out = dram.tile([128, 1], F32)
                nc.gpsimd.dma_start(sb_in[:], slc[:])
                nc.gpsimd.collective_compute(
                    "AllReduce", ALU.add,
                    replica_groups=[list(range(c.ncore))],
                    ins=[sb_in.opt()], outs=[sb_out.opt()],
                )
                st = wk.tile([128, 1], F32, tag="st")
                nc.sync.dma_start(out=st[:], in_=sb_out[:])
                # fold [128,1] -> [64,1]: total expsum per batch
                stf_ps = psF.tile([c.B, 1], F32, space="PSUM", tag="stf")
                nc.tensor.matmul(out=stf_ps[:], lhsT=wfold[:], rhs=st[:],
                                 start=True, stop=True)
                lseS = wk.tile([c.B, 1], F32, tag="lseS")
                nc.scalar.activation(out=lseS[:], in_=stf_ps[:], func=AF.Ln)
                # unfold [64,1] -> [128,1] and pre-shift by -ln(V)
                lse2_ps = psF.tile([128, 1], F32, space="PSUM", tag="lse2")
                nc.tensor.matmul(out=lse2_ps[:], lhsT=wunf[:], rhs=lseS[:],
                                 start=True, stop=True)
                lse2 = wk.tile([128, 1], F32, tag="lse2s")
                nc.vector.tensor_scalar(out=lse2[:], in0=lse2_ps[:],
                                        scalar1=-LOGV_SHIFT, scalar2=None,
                                        op0=ALU.add)
                # subtract + store output, in halves (overlap DVE with DMA)
                H = VSH // 4
                for i in range(2):
                    sl = slice(i * H, (i + 1) * H if i == 0 else VSH // 2)
                    nc.vector.tensor_scalar(out=z128[:, sl], in0=z128[:, sl],
                                            scalar1=lse2[:], scalar2=None,
                                            op0=ALU.subtract)
                    nc.sync.dma_start(out=t_o.ap()[:, sl], in_=z128[:, sl])

    nc.compile()
    return nc


def host_prep(cfg, x, q, A, TA, TC):
    c = cfg
    E, J, S = c.E, c.J, c.S
    x = np.asarray(x).astype(np.int64)
    q = np.asarray(q).astype(np.int64)
    A = np.asarray(A, dtype=np.float32)
    TA = np.asarray(TA, dtype=np.float32)
    TC = np.asarray(TC, dtype=np.float32)

    tabI = np.ascontiguousarray(A.transpose(1, 0, 2).reshape(c.V, 4 * E))
    tabI = tabI.astype(c.npdt)
    a3tF = np.ascontiguousarray(A[3].T)  # [E, V] f32

    j = np.arange(1, J + 1, dtype=np.float32)
    av = 1.0 - j / J
    bv = 2.0 * j / J - 1.0
    sp = np.arange(128) // J
    jj = np.arange(128) % J
    wab = np.zeros((128, 2 * c.SPT), np.float32)
    wabc = np.zeros((128, 3 * c.SPT), np.float32)
    wc1 = np.zeros((128, c.SPT), np.float32)
    for p in range(128):
        wab[p, 2 * sp[p] + 0] = av[jj[p]]
        wab[p, 2 * sp[p] + 1] = bv[jj[p]]
        wabc[p, 3 * sp[p] + 0] = av[jj[p]]
        wabc[p, 3 * sp[p] + 1] = bv[jj[p]]
        wabc[p, 3 * sp[p] + 2] = 1.0
        wc1[p, sp[p]] = 1.0
    wq = np.zeros((128, c.Bc), np.float32)
    for p in range(128):
        wq[p, p // c.QW] = 1.0

    tat = np.ascontiguousarray(TA[0, :S, :].T)          # [E, S] f32
    tct = np.ascontiguousarray(TC[0, :S, :].T)          # [E, S] f32
    kp = ((np.arange(E, dtype=np.float32) + 1.0) / E).reshape(E, 1)
    id1 = np.ones((1, 1), np.float32)
    id128 = np.eye(128, dtype=np.float32)
    wfold = np.zeros((128, c.B), np.float32)
    for p in range(128):
        wfold[p, p % c.B] = 1.0
    wunf = np.zeros((c.B, 128), np.float32)
    for b in range(c.B):
        wunf[b, b] = 1.0
        wunf[b, c.B + b] = 1.0

    common = {
        "wab": wab.astype(c.npdt), "wabc": wabc.astype(c.npdt),
        "wc1": wc1.astype(c.npdt),
        "wq": wq.astype(c.npdt), "tat": tat, "tct": tct,
        "kp": kp, "id1": id1, "id128": id128.astype(c.npdt),
        "wfold": wfold, "wunf": wunf,
    }

    in_maps = []
    for cc in range(c.ncore):
        qc = q[cc * c.Bc:(cc + 1) * c.Bc].reshape(-1)
        xc = x[cc * c.Bc:(cc + 1) * c.Bc].reshape(c.Bc, -1)
        xq = np.concatenate([qc] + [xc[b] for b in range(c.Bc)])
        uniq, rel = np.unique(xq, return_inverse=True)
        assert len(uniq) <= c.ucap, (len(uniq), c.ucap)
        tabc = np.zeros((c.ucap, 4 * E), c.npdt)
        tabc[:len(uniq)] = tabI[uniq]
        rel = rel.astype(np.int16)
        idx = np.zeros((128, c.TOT16), np.int16)
        off = 0
        for gs in c.gsizes:
            v = rel[off:off + gs]
            wrapped = v.reshape(-1, 16).T
            idx[:, off // 16:(off + gs) // 16] = np.tile(wrapped, (8, 1))
            off += gs
        a3c = np.ascontiguousarray(
            a3tF[:, cc * c.VSH:(cc + 1) * c.VSH]).astype(c.znp)
        m = dict(common)
        m.update({"tabc": tabc, "idx": idx, "a3t": a3c})
        in_maps.append(m)
    return in_maps


_CACHE = {}


def _get_module(cfg):
    k = cfg.key()
    if k not in _CACHE:
        _CACHE[k] = build_module(cfg)
    return _CACHE[k]


def run(cfg, inputs, trace=False):
    nc = _get_module(cfg)
    in_maps = host_prep(cfg, inputs["x"], inputs["q"], inputs["A"],
                        inputs["TA"], inputs["TC"])
    res = bass_utils.run_bass_kernel_spmd(
        nc, in_maps, core_ids=list(range(cfg.ncore)), trace=trace)
    parts = []
    for cc in range(cfg.ncore):
        z = np.asarray(res.results[cc]["o"]).astype(np.float32)
        # [128, VSH/2] -> [64, VSH]: partition p = half*64 + b,
        # col j = k*zh + c  ->  out[b, k*zc + half*zh + c]
        z = z.reshape(2, cfg.B, cfg.nzc, cfg.zh)
        z = z.transpose(1, 2, 0, 3).reshape(cfg.B, cfg.VSH)
        parts.append(z)
    out = np.concatenate(parts, axis=1)
    return out, res


def kernel(**inputs) -> np.ndarray:
    cfg = Cfg()
    out, _ = run(cfg, inputs, trace=False)
    return out


# revision 28
# speedup vs baseline: 1.1173x; 1.0108x over previous
"""MemNN (embedding_lookup) Trainium2 Bass kernel — v2.

Strategy (8 NeuronCores, one NEFF, SPMD):
  - Data-parallel: batch dim sharded 8 ways (8 batches/core).
  - Host packs the 4 embedding tables interleaved per vocab row
    ([A0|A1|A2|A3][v], bf16) and, per core, compacts it to the core's
    unique vocab rows so indices fit dma_gather's int16.
  - Token order: queries first, then batch-major stories.  Each gather
    chunk covers exactly one batch (last batch split into shrinking
    sub-chunks to minimize end-of-pipeline exposure).  GpSimd descriptor
    generation (~8ns/row) is the critical path; everything else is
    scheduled under it:
      * PE reduces each 128-row tile into per-sentence partial sums:
        m-path via rank-2-separable position encoding
        (pe[j,d] = a(j) + b(j)*k'(d)) into [E, sent] layout; c-path via
        swapped-operand matmuls directly into [sent, E] layout (no
        transposes later).  TC is folded into the hop-update matmul as a
        constant lhsT (sum_s p_s * tc_s).
      * The 3 attention hops run per batch as soon as that batch's
        chunks land — hidden under the remaining gathers.
  - AllGather u across cores; vocab-sharded logits z = u @ A3^T in
    [128, VSH/2] layout (batch-pairs on partitions), exp-sums off PSUM,
    AllReduce, log_softmax; bf16 output with a -ln(V) pre-shift to keep
    bf16 rounding small.
"""

import numpy as np
import ml_dtypes

import concourse.bass as bass
import concourse.mybir as mybir
import concourse.tile as tile
from concourse import bacc
import concourse.bass_utils as bass_utils

F32 = mybir.dt.float32
AF = mybir.ActivationFunctionType
ALU = mybir.AluOpType
AX = mybir.AxisListType

LOGV_SHIFT = float(np.log(100000.0))


class Cfg:
    def __init__(self, ncore=8, B=64, S=50, J=64, QW=16, V=100000, E=128,
                 ucap=24576, gchunk=1024, use_bf16=True, z_f32=False):
        self.ncore, self.B, self.S, self.J, self.QW = ncore, B, S, J, QW
        self.V, self.E, self.ucap = V, E, ucap
        self.use_bf16, self.z_f32 = use_bf16, z_f32
        self.Bc = B // ncore
        self.NQ = self.Bc * QW             # query rows per core
        assert self.NQ == 128
        self.TPB = S * J                   # story tokens per batch
        assert self.TPB % 128 == 0
        self.tiles_pb = self.TPB // 128    # 128-row tiles per batch
        self.SPT = 128 // J                # sentences per tile
        assert 128 % J == 0 and self.SPT == 2
        self.NPOS = self.NQ + self.Bc * self.TPB
        self.VSH = V // ncore
        # chunks: (batch, tile_offset_within_batch, ntiles); batch -1 = query
        # dma_gather chunks are capped at 8 tiles (1024 rows) — larger
        # num_idxs hangs on HW (works in CoreSim; descriptor-ring capacity
        # is not simulated).
        self.gt_max = 8
        chunks = [(-1, 0, 1)]
        for b in range(self.Bc):
            t = 0
            while t < self.tiles_pb:
                nt = min(self.gt_max, self.tiles_pb - t)
                chunks.append((b, t, nt))
                t += nt
        self.chunks = chunks
        self.gsizes = [nt * 128 for (_, _, nt) in chunks]
        assert sum(self.gsizes) == self.NPOS
        self.TOT16 = self.NPOS // 16
        self.DT = mybir.dt.bfloat16 if use_bf16 else mybir.dt.float32
        self.npdt = ml_dtypes.bfloat16 if use_bf16 else np.float32
        self.zdt = F32 if z_f32 else self.DT
        self.znp = np.float32 if z_f32 else self.npdt
        # logits column chunking
        self.zc = 500 if self.VSH % 500 == 0 else self.VSH
        assert self.VSH % self.zc == 0 and self.zc % 2 == 0
        self.nzc = self.VSH // self.zc
        self.zh = self.zc // 2             # half-chunk (partition fold)

    def key(self):
        return (self.ncore, self.B, self.S, self.J, self.QW, self.V, self.E,
                self.ucap, self.use_bf16, self.z_f32)


def build_module(cfg):
    c = cfg
    E, Bc, S, VSH = c.E, c.Bc, c.S, c.VSH
    DT = c.DT
    nc = bacc.Bacc("TRN2", target_bir_lowering=False, debug=False,
                   num_devices=c.ncore, num_swdge_queues=1)

    t_idx = nc.dram_tensor("idx", [128, c.TOT16], mybir.dt.int16,
                           kind="ExternalInput")
    t_tab = nc.dram_tensor("tabc", [c.ucap, 4 * E], DT, kind="ExternalInput")
    # all small constants packed into two tensors (one DMA each):
    # cb (DT): wab[0:4] wabc[4:10] wc1[10:12] wq[12:12+Bc] id128[+128]
    # cf (F32): tat[0:S] tct[S:2S] kp[2S] id1[2S+1] wfold[+B] wunf[+128]
    NB = 2 * c.SPT + 3 * c.SPT + c.SPT + Bc + 128
    NF = 2 * S + 2 + c.B + 128
    t_cb = nc.dram_tensor("cb", [128, NB], DT, kind="ExternalInput")
    t_cf = nc.dram_tensor("cf", [128, NF], F32, kind="ExternalInput")
    assert 2 * c.B == 128  # output fold uses 128 partitions = 2*B
    t_a3t = nc.dram_tensor("a3t", [E, VSH], c.zdt, kind="ExternalInput")

    t_o = nc.dram_tensor("o", [2 * c.B, VSH // 2], c.zdt,
                         kind="ExternalOutput")     # [128, VSH/2]
    t_du = nc.dram_tensor("du", [E, c.B], F32, kind="ExternalOutput")

    with tile.TileContext(nc) as tc:
        with tc.tile_pool(name="const", bufs=1) as cpool, \
             tc.tile_pool(name="gp", bufs=3) as gpool, \
             tc.tile_pool(name="wk", bufs=1) as wk, \
             tc.tile_pool(name="hp", bufs=3) as hp, \
             tc.tile_pool(name="big", bufs=1) as big, \
             tc.tile_pool(name="dram", bufs=1, space="DRAM") as dram:

            # ---- constant loads (idx first: gates the first gather) -------
            idxs = cpool.tile([128, c.TOT16], mybir.dt.int16)
            nc.sync.dma_start(out=idxs[:], in_=t_idx.ap())
            cb = cpool.tile([128, NB], DT)
            nc.sync.dma_start(out=cb[:], in_=t_cb.ap())
            cf = cpool.tile([128, NF], F32)
            nc.sync.dma_start(out=cf[:], in_=t_cf.ap())
            o = 0
            wab = cb[:, o:o + 2 * c.SPT]; o += 2 * c.SPT
            wabc = cb[:, o:o + 3 * c.SPT]; o += 3 * c.SPT
            wc1 = cb[:, o:o + c.SPT]; o += c.SPT
            wq = cb[:, o:o + Bc]; o += Bc
            id128 = cb[:, o:o + 128]
            o = 0
            tat = cf[:, o:o + S]; o += S
            tct = cf[:, o:o + S]; o += S
            kp = cf[:, o:o + 1]; o += 1
            id1 = cf[0:1, o:o + 1]; o += 1
            wfold = cf[:, o:o + c.B]; o += c.B
            wunf = cf[0:c.B, o:o + 128]
            a3t = big.tile([E, VSH], c.zdt)
            nc.sync.dma_start(out=a3t[:], in_=t_a3t.ap())

            # ---- persistent work tiles ------------------------------------
            mst = [wk.tile([E, 3, S], F32, tag=f"mst{b}", name=f"mst{b}")
                   for b in range(Bc)]
            cstE = [wk.tile([E, 3, S], DT, tag=f"cstE{b}", name=f"cstE{b}")
                    for b in range(Bc)]
            cst = [wk.tile([S, 3 * E], DT, tag=f"cst{b}", name=f"cst{b}")
                   for b in range(Bc)]
            uS = [wk.tile([E, Bc], F32, tag=f"uS{i}", name=f"uS{i}")
                  for i in range(4)]

            with tc.tile_pool(name="psM", bufs=3, space="PSUM") as psM, \
                 tc.tile_pool(name="psC", bufs=2, space="PSUM") as psC, \
                 tc.tile_pool(name="psH", bufs=1, space="PSUM") as psH:

                def hop_stage(b, h):
                    # softmax without max-subtraction: scores are bounded
                    # (|score| < 60 for this model family; exp is f32-safe)
                    sc_ps = psH.tile([1, S], F32, space="PSUM", tag="sc")
                    nc.tensor.matmul(out=sc_ps[:],
                                     lhsT=uS[h][:, b:b + 1],
                                     rhs=mst[b][:, h, :],
                                     start=True, stop=True)
                    ex = hp.tile([1, S], F32, tag="ex")
                    zsum = hp.tile([1, 1], F32, tag="zsum")
                    nc.scalar.activation(out=ex[:], in_=sc_ps[:],
                                         func=AF.Exp, accum_out=zsum[:])
                    rz = hp.tile([1, 1], F32, tag="rz")
                    nc.vector.reciprocal(out=rz[:], in_=zsum[:])
                    exn = hp.tile([1, S], F32, tag="exn")
                    nc.vector.tensor_scalar(out=exn[:], in0=ex[:],
                                            scalar1=rz[:], scalar2=None,
                                            op0=ALU.mult)
                    pt_ps = psH.tile([S, 1], F32, space="PSUM", tag="pt")
                    nc.tensor.transpose(out=pt_ps[:], in_=exn[:],
                                        identity=id1)
                    pt = hp.tile([S, 1], DT, tag="ptb")
                    nc.vector.tensor_copy(pt[:], pt_ps[:])
                    up_ps = psH.tile([E, 1], F32, space="PSUM", tag="up")
                    nc.tensor.matmul(out=up_ps[:],
                                     lhsT=cst[b][:, h * E:(h + 1) * E],
                                     rhs=pt[:], start=True, stop=True)
                    nc.vector.tensor_tensor(out=uS[h + 1][:, b:b + 1],
                                            in0=up_ps[:],
                                            in1=uS[h][:, b:b + 1],
                                            op=ALU.add)

                def tr_stage(b):
                    # transpose c to [sent, E] for the update matmuls
                    for h in range(3):
                        cn_ps = psC.tile([S, E], DT, space="PSUM", tag="cn")
                        nc.tensor.transpose(out=cn_ps[:],
                                            in_=cstE[b][:, h, :],
                                            identity=id128)
                        nc.vector.tensor_copy(
                            cst[b][:, h * E:(h + 1) * E], cn_ps[:])

                # ---- gather chunks + reductions + per-batch hops ----------
                # Hop/transpose stages are drained one per chunk so their
                # sem waits sit between chunk matmul bursts on each
                # sequencer (no head-of-line blocking).
                import collections
                stages = collections.deque()
                L0, L1, L2, L3 = 0, 2 * S, 5 * S, 8 * S
                off16 = 0
                for (b, t0, ntiles) in c.chunks:
                    gs = ntiles * 128
                    if b < 0:
                        # query chunk
                        gq = cpool.tile([128, 1, 4 * E], DT)
                        nc.gpsimd.dma_gather(
                            out_ap=gq[:, :1, :], in_ap=t_tab.ap(),
                            idxs_ap=idxs[:, off16:off16 + gs // 16],
                            num_idxs=gs, num_idxs_reg=gs, elem_size=4 * E)
                        Pq = psM.tile([128, 9 * S], F32, space="PSUM",
                                      tag="Pm")
                        nc.tensor.matmul(out=Pq[:, 0:Bc],
                                         lhsT=gq[:, 0, 0:E], rhs=wq,
                                         start=True, stop=True)
                        nc.vector.tensor_copy(uS[0][:], Pq[:, 0:Bc])
                        off16 += gs // 16
                        continue

                    spc = ntiles * c.SPT       # sentences in this chunk
                    s0 = t0 * c.SPT            # batch-local sentence offset
                    gt = gpool.tile([128, c.gt_max, 4 * E], DT, tag="g")
                    nc.gpsimd.dma_gather(
                        out_ap=gt[:, :ntiles, :], in_ap=t_tab.ap(),
                        idxs_ap=idxs[:, off16:off16 + gs // 16],
                        num_idxs=gs, num_idxs_reg=gs, elem_size=4 * E)
                    off16 += gs // 16

                    Pm = psM.tile([128, 9 * S], F32, space="PSUM", tag="Pm")
                    for t in range(ntiles):
                        sl = 2 * t             # chunk-local sentence base
                        nc.tensor.matmul(
                            out=Pm[:, L0 + 2 * sl:L0 + 2 * sl + 4],
                            lhsT=gt[:, t, 0:E], rhs=wab,
                            start=True, stop=True)
                        nc.tensor.matmul(
                            out=Pm[:, L1 + 3 * sl:L1 + 3 * sl + 6],
                            lhsT=gt[:, t, E:2 * E], rhs=wabc,
                            start=True, stop=True)
                        nc.tensor.matmul(
                            out=Pm[:, L2 + 3 * sl:L2 + 3 * sl + 6],
                            lhsT=gt[:, t, 2 * E:3 * E], rhs=wabc,
                            start=True, stop=True)
                        nc.tensor.matmul(
                            out=Pm[:, L3 + sl:L3 + sl + 2],
                            lhsT=gt[:, t, 3 * E:4 * E], rhs=wc1,
                            start=True, stop=True)

                    # combines: m = Sa + kp*Sb + tat ; cE = Csum + tct
                    pap = Pm[:]
                    pdim = pap.ap[0]

                    def pv(off, stride):
                        return bass.AP(pap.tensor, pap.offset + off,
                                       [pdim, (stride, spc)])

                    for h, (sa, sb) in enumerate(
                            [(pv(L0, 2), pv(L0 + 1, 2)),
                             (pv(L1, 3), pv(L1 + 1, 3)),
                             (pv(L2, 3), pv(L2 + 1, 3))]):
                        msl = mst[b][:, h, s0:s0 + spc]
                        nc.vector.tensor_scalar(out=msl, in0=sb,
                                                scalar1=kp, scalar2=None,
                                                op0=ALU.mult)
                        nc.vector.tensor_tensor(out=msl, in0=msl, in1=sa,
                                                op=ALU.add)
                        nc.vector.tensor_tensor(out=msl, in0=msl,
                                                in1=tat[:, s0:s0 + spc],
                                                op=ALU.add)
                    for h, cv in enumerate([pv(L1 + 2, 3), pv(L2 + 2, 3),
                                            pv(L3, 1)]):
                        nc.vector.tensor_tensor(
                            out=cstE[b][:, h, s0:s0 + spc], in0=cv,
                            in1=tct[:, s0:s0 + spc], op=ALU.add)

                    if t0 + ntiles == c.tiles_pb:
                        stages.append(lambda bb=b: tr_stage(bb))
                        for h in range(3):
                            stages.append(lambda bb=b, hh=h:
                                          hop_stage(bb, hh))
                    if stages:
                        stages.popleft()()

                while stages:
                    stages.popleft()()

            # ---- AllGather u ----------------------------------------------
            ub_in = dram.tile([E, Bc], F32)
            ub_out = dram.tile([c.ncore * E, Bc], F32)
            nc.gpsimd.dma_start(ub_in[:], uS[3][:])
            nc.gpsimd.collective_compute(
                "AllGather", ALU.bypass,
                replica_groups=[list(range(c.ncore))],
                ins=[ub_in.opt()], outs=[ub_out.opt()],
            )
            uTf = wk.tile([E, c.ncore, Bc], F32, tag="uTf")
            src = bass.AP(ub_out[:].tensor, ub_out[:].offset,
                          [(Bc, E), (E * Bc, c.ncore), (1, Bc)])
            nc.sync.dma_start(out=uTf[:], in_=src)
            nc.sync.dma_start(out=t_du.ap(),
                              in_=uTf[:].rearrange("e c b -> e (c b)"))
            uz = wk.tile([E, c.B], c.zdt, tag="uz")
            nc.vector.tensor_copy(uz[:], uTf[:].rearrange("e c b -> e (c b)"))

            # ---- logits + log_softmax -------------------------------------
            z128 = big.tile([128, VSH // 2], c.zdt)
            sums = wk.tile([128, c.nzc], F32, tag="sums")
            with tc.tile_pool(name="psZ", bufs=4, space="PSUM") as psZ, \
                 tc.tile_pool(name="psF", bufs=1, space="PSUM") as psF:
                for k in range(c.nzc):
                    zps = psZ.tile([128, c.zh], F32, space="PSUM", tag="z")
                    nc.tensor.matmul(out=zps[0:c.B, :], lhsT=uz[:],
                                     rhs=a3t[:, k * c.zc:k * c.zc + c.zh],
                                     start=True, stop=True)
                    nc.tensor.matmul(out=zps[c.B:2 * c.B, :], lhsT=uz[:],
                                     rhs=a3t[:, k * c.zc + c.zh:
                                             (k + 1) * c.zc],
                                     start=True, stop=True)
                    nc.vector.tensor_scalar(
                        out=z128[:, k * c.zh:(k + 1) * c.zh], in0=zps[:],
                        scalar1=-LOGV_SHIFT, scalar2=None, op0=ALU.add)
                    esc = hp.tile([128, c.zh], DT, tag="esc")
                    nc.scalar.activation(out=esc[:], in_=zps[:], func=AF.Exp,
                                         accum_out=sums[:, k:k + 1])

                slc = wk.tile([128, 1], F32, tag="slc")
                nc.vector.tensor_reduce(out=slc[:], in_=sums[:], axis=AX.X,
                                        op=ALU.add)
                sb_in = dram.tile([128, 1], F32)
                sb_out = dram.tile([128, 1], F32)
                nc.gpsimd.dma_start(sb_in[:], slc[:])
                nc.gpsimd.collective_compute(
                    "AllReduce", ALU.add,
                    replica_groups=[list(range(c.ncore))],
                    ins=[sb_in.opt()], outs=[sb_out.opt()],
                )
                st = wk.tile([128, 1], F32, tag="st")
                nc.sync.dma_start(out=st[:], in_=sb_out[:])
                # fold [128,1] -> [64,1]: total expsum per batch
                stf_ps = psF.tile([c.B, 1], F32, space="PSUM", tag="stf")
                nc.tensor.matmul(out=stf_ps[:], lhsT=wfold, rhs=st[:],
                                 start=True, stop=True)
                lseS = wk.tile([c.B, 1], F32, tag="lseS")
                nc.scalar.activation(out=lseS[:], in_=stf_ps[:], func=AF.Ln)
                # unfold [64,1] -> [128,1] and pre-shift by -ln(V)
                lse2_ps = psF.tile([128, 1], F32, space="PSUM", tag="lse2")
                nc.tensor.matmul(out=lse2_ps[:], lhsT=wunf, rhs=lseS[:],
                                 start=True, stop=True)
                lse2 = wk.tile([128, 1], F32, tag="lse2s")
                nc.vector.tensor_scalar(out=lse2[:], in0=lse2_ps[:],
                                        scalar1=-LOGV_SHIFT, scalar2=None,
                                        op0=ALU.add)
                # subtract + store output, in halves (overlap DVE with DMA)
                H = VSH // 4
                for i in range(2):
                    sl = slice(i * H, (i + 1) * H if i == 0 else VSH // 2)
                    nc.vector.tensor_scalar(out=z128[:, sl], in0=z128[:, sl],
                                            scalar1=lse2[:], scalar2=None,
                                            op0=ALU.subtract)
                    nc.sync.dma_start(out=t_o.ap()[:, sl], in_=z128[:, sl])

    nc.compile()
    return nc


def host_prep(cfg, x, q, A, TA, TC):
    c = cfg
    E, J, S = c.E, c.J, c.S
    x = np.asarray(x).astype(np.int64)
    q = np.asarray(q).astype(np.int64)
    A = np.asarray(A, dtype=np.float32)
    TA = np.asarray(TA, dtype=np.float32)
    TC = np.asarray(TC, dtype=np.float32)

    tabI = np.ascontiguousarray(A.transpose(1, 0, 2).reshape(c.V, 4 * E))
    tabI = tabI.astype(c.npdt)
    a3tF = np.ascontiguousarray(A[3].T)  # [E, V] f32

    j = np.arange(1, J + 1, dtype=np.float32)
    av = 1.0 - j / J
    bv = 2.0 * j / J - 1.0
    sp = np.arange(128) // J
    jj = np.arange(128) % J
    wab = np.zeros((128, 2 * c.SPT), np.float32)
    wabc = np.zeros((128, 3 * c.SPT), np.float32)
    wc1 = np.zeros((128, c.SPT), np.float32)
    for p in range(128):
        wab[p, 2 * sp[p] + 0] = av[jj[p]]
        wab[p, 2 * sp[p] + 1] = bv[jj[p]]
        wabc[p, 3 * sp[p] + 0] = av[jj[p]]
        wabc[p, 3 * sp[p] + 1] = bv[jj[p]]
        wabc[p, 3 * sp[p] + 2] = 1.0
        wc1[p, sp[p]] = 1.0
    wq = np.zeros((128, c.Bc), np.float32)
    for p in range(128):
        wq[p, p // c.QW] = 1.0

    tat = np.ascontiguousarray(TA[0, :S, :].T)          # [E, S] f32
    tct = np.ascontiguousarray(TC[0, :S, :].T)          # [E, S] f32
    kp = ((np.arange(E, dtype=np.float32) + 1.0) / E).reshape(E, 1)
    id1 = np.ones((1, 1), np.float32)
    id128 = np.eye(128, dtype=np.float32)
    wfold = np.zeros((128, c.B), np.float32)
    for p in range(128):
        wfold[p, p % c.B] = 1.0
    wunf = np.zeros((c.B, 128), np.float32)
    for b in range(c.B):
        wunf[b, b] = 1.0
        wunf[b, c.B + b] = 1.0

    cb = np.concatenate([wab, wabc, wc1, wq, id128], axis=1).astype(c.npdt)
    id1f = np.zeros((128, 1), np.float32)
    id1f[0, 0] = 1.0
    wunf128 = np.zeros((128, 128), np.float32)
    wunf128[:c.B, :] = wunf
    cf = np.concatenate(
        [tat, tct, kp, id1f, wfold, wunf128], axis=1).astype(np.float32)
    common = {"cb": cb, "cf": cf}

    in_maps = []
    for cc in range(c.ncore):
        qc = q[cc * c.Bc:(cc + 1) * c.Bc].reshape(-1)
        xc = x[cc * c.Bc:(cc + 1) * c.Bc].reshape(c.Bc, -1)
        xq = np.concatenate([qc] + [xc[b] for b in range(c.Bc)])
        uniq, rel = np.unique(xq, return_inverse=True)
        assert len(uniq) <= c.ucap, (len(uniq), c.ucap)
        tabc = np.zeros((c.ucap, 4 * E), c.npdt)
        tabc[:len(uniq)] = tabI[uniq]
        rel = rel.astype(np.int16)
        idx = np.zeros((128, c.TOT16), np.int16)
        off = 0
        for gs in c.gsizes:
            v = rel[off:off + gs]
            wrapped = v.reshape(-1, 16).T
            idx[:, off // 16:(off + gs) // 16] = np.tile(wrapped, (8, 1))
            off += gs
        a3c = np.ascontiguousarray(
            a3tF[:, cc * c.VSH:(cc + 1) * c.VSH]).astype(c.znp)
        m = dict(common)
        m.update({"tabc": tabc, "idx": idx, "a3t": a3c})
        in_maps.append(m)
    return in_maps


_CACHE = {}


def _get_module(cfg):
    k = cfg.key()
    if k not in _CACHE:
        _CACHE[k] = build_module(cfg)
    return _CACHE[k]


def run(cfg, inputs, trace=False):
    nc = _get_module(cfg)
    in_maps = host_prep(cfg, inputs["x"], inputs["q"], inputs["A"],
                        inputs["TA"], inputs["TC"])
    res = bass_utils.run_bass_kernel_spmd(
        nc, in_maps, core_ids=list(range(cfg.ncore)), trace=trace)
    parts = []
    for cc in range(cfg.ncore):
        z = np.asarray(res.results[cc]["o"]).astype(np.float32)
        # [128, VSH/2] -> [64, VSH]: partition p = half*64 + b,
        # col j = k*zh + c  ->  out[b, k*zc + half*zh + c]
        z = z.reshape(2, cfg.B, cfg.nzc, cfg.zh)
        z = z.transpose(1, 2, 0, 3).reshape(cfg.B, cfg.VSH)
        parts.append(z)
    out = np.concatenate(parts, axis=1)
    return out, res


def kernel(**inputs) -> np.ndarray:
    cfg = Cfg()
    out, _ = run(cfg, inputs, trace=False)
    return out


# revision 44
# speedup vs baseline: 1.1409x; 1.0211x over previous
"""MemNN (embedding_lookup) Trainium2 Bass kernel — v2.

Strategy (8 NeuronCores, one NEFF, SPMD):
  - Data-parallel: batch dim sharded 8 ways (8 batches/core).
  - Host packs the 4 embedding tables interleaved per vocab row
    ([A0|A1|A2|A3][v], bf16) and, per core, compacts it to the core's
    unique vocab rows so indices fit dma_gather's int16.
  - Token order: queries first, then batch-major stories.  Each gather
    chunk covers exactly one batch (last batch split into shrinking
    sub-chunks to minimize end-of-pipeline exposure).  GpSimd descriptor
    generation (~8ns/row) is the critical path; everything else is
    scheduled under it:
      * PE reduces each 128-row tile into per-sentence partial sums:
        m-path via rank-2-separable position encoding
        (pe[j,d] = a(j) + b(j)*k'(d)) into [E, sent] layout; c-path via
        swapped-operand matmuls directly into [sent, E] layout (no
        transposes later).  TC is folded into the hop-update matmul as a
        constant lhsT (sum_s p_s * tc_s).
      * The 3 attention hops run per batch as soon as that batch's
        chunks land — hidden under the remaining gathers.
  - AllGather u across cores; vocab-sharded logits z = u @ A3^T in
    [128, VSH/2] layout (batch-pairs on partitions), exp-sums off PSUM,
    AllReduce, log_softmax; bf16 output with a -ln(V) pre-shift to keep
    bf16 rounding small.
"""

import numpy as np
import ml_dtypes

import concourse.bass as bass
import concourse.mybir as mybir
import concourse.tile as tile
from concourse import bacc
import concourse.bass_utils as bass_utils

F32 = mybir.dt.float32
AF = mybir.ActivationFunctionType
ALU = mybir.AluOpType
AX = mybir.AxisListType

LOGV_SHIFT = float(np.log(100000.0))


class Cfg:
    def __init__(self, ncore=8, B=64, S=50, J=64, QW=16, V=100000, E=128,
                 ucap=24576, gchunk=1024, use_bf16=True, z_f32=False):
        self.ncore, self.B, self.S, self.J, self.QW = ncore, B, S, J, QW
        self.V, self.E, self.ucap = V, E, ucap
        self.use_bf16, self.z_f32 = use_bf16, z_f32
        self.Bc = B // ncore
        self.NQ = self.Bc * QW             # query rows per core
        assert self.NQ == 128
        self.TPB = S * J                   # story tokens per batch
        assert self.TPB % 128 == 0
        self.tiles_pb = self.TPB // 128    # 128-row tiles per batch
        self.SPT = 128 // J                # sentences per tile
        assert 128 % J == 0 and self.SPT == 2
        self.NPOS = self.NQ + self.Bc * self.TPB
        self.VSH = V // ncore
        # chunks: (batch, tile_offset_within_batch, ntiles); batch -1 = query
        # dma_gather chunks are capped at 8 tiles (1024 rows) — larger
        # num_idxs hangs on HW (works in CoreSim; descriptor-ring capacity
        # is not simulated).
        self.gt_max = 8
        chunks = [(-1, 0, 1)]
        for b in range(self.Bc):
            t = 0
            while t < self.tiles_pb:
                nt = min(self.gt_max, self.tiles_pb - t)
                chunks.append((b, t, nt))
                t += nt
        self.chunks = chunks
        self.gsizes = [nt * 128 for (_, _, nt) in chunks]
        assert sum(self.gsizes) == self.NPOS
        self.TOT16 = self.NPOS // 16
        self.DT = mybir.dt.bfloat16 if use_bf16 else mybir.dt.float32
        self.npdt = ml_dtypes.bfloat16 if use_bf16 else np.float32
        self.zdt = F32 if z_f32 else self.DT
        self.znp = np.float32 if z_f32 else self.npdt
        # logits column chunking: chunks of up to 1024 cols (two <=512
        # matmuls each), even widths so the partition fold halves cleanly
        self.zws = []
        rem = self.VSH
        while rem > 0:
            w = min(1024, rem)
            if rem - w > 0 and rem - w < 4:
                w -= 4
            self.zws.append(w)
            rem -= w
        assert all(w % 2 == 0 and w // 2 <= 512 for w in self.zws)
        self.nzc = len(self.zws)

    def key(self):
        return (self.ncore, self.B, self.S, self.J, self.QW, self.V, self.E,
                self.ucap, self.use_bf16, self.z_f32)


def build_module(cfg):
    c = cfg
    E, Bc, S, VSH = c.E, c.Bc, c.S, c.VSH
    DT = c.DT
    nc = bacc.Bacc("TRN2", target_bir_lowering=False, debug=False,
                   num_devices=c.ncore, num_swdge_queues=1)

    t_idx = nc.dram_tensor("idx", [128, c.TOT16], mybir.dt.int16,
                           kind="ExternalInput")
    t_tab = nc.dram_tensor("tabc", [c.ucap, 4 * E], DT, kind="ExternalInput")
    # all small constants packed into two tensors (one DMA each):
    # cb (DT): wab[0:4] wabc[4:10] wc1[10:12] wq[12:12+Bc] id128[+128]
    # cf (F32): tat[0:S] tct[S:2S] kp[2S] id1[2S+1] wfold[+B] wunf[+128]
    NB = 2 * c.SPT + 3 * c.SPT + c.SPT + Bc + 128
    NF = 2 * S + 2 + c.B + 128
    t_cb = nc.dram_tensor("cb", [128, NB], DT, kind="ExternalInput")
    t_cf = nc.dram_tensor("cf", [128, NF], F32, kind="ExternalInput")
    assert 2 * c.B == 128  # output fold uses 128 partitions = 2*B
    t_a3t = nc.dram_tensor("a3t", [E, VSH], c.zdt, kind="ExternalInput")

    t_o = nc.dram_tensor("o", [2 * c.B, VSH // 2], c.zdt,
                         kind="ExternalOutput")     # [128, VSH/2]
    t_du = nc.dram_tensor("du", [E, c.B], F32, kind="ExternalOutput")

    with tile.TileContext(nc) as tc:
        with tc.tile_pool(name="const", bufs=1) as cpool, \
             tc.tile_pool(name="gp", bufs=3) as gpool, \
             tc.tile_pool(name="wk", bufs=1) as wk, \
             tc.tile_pool(name="hp", bufs=3) as hp, \
             tc.tile_pool(name="big", bufs=1) as big, \
             tc.tile_pool(name="dram", bufs=1, space="DRAM") as dram:

            # ---- constant loads (idx first: gates the first gather) -------
            idxs = cpool.tile([128, c.TOT16], mybir.dt.int16)
            nc.sync.dma_start(out=idxs[:], in_=t_idx.ap())
            cb = cpool.tile([128, NB], DT)
            nc.sync.dma_start(out=cb[:], in_=t_cb.ap())
            cf = cpool.tile([128, NF], F32)
            nc.sync.dma_start(out=cf[:], in_=t_cf.ap())
            o = 0
            wab = cb[:, o:o + 2 * c.SPT]; o += 2 * c.SPT
            wabc = cb[:, o:o + 3 * c.SPT]; o += 3 * c.SPT
            wc1 = cb[:, o:o + c.SPT]; o += c.SPT
            wq = cb[:, o:o + Bc]; o += Bc
            id128 = cb[:, o:o + 128]
            o = 0
            tat = cf[:, o:o + S]; o += S
            tct = cf[:, o:o + S]; o += S
            kp = cf[:, o:o + 1]; o += 1
            id1 = cf[0:1, o:o + 1]; o += 1
            wfold = cf[:, o:o + c.B]; o += c.B
            wunf = cf[0:c.B, o:o + 128]
            # a3t rides the scalar engine's DMA queue so the idx-gated
            # first gather never queues behind this 3.2MB transfer
            a3t = big.tile([E, VSH], c.zdt)
            nc.scalar.dma_start(out=a3t[:], in_=t_a3t.ap())

            # ---- persistent work tiles ------------------------------------
            mst = [wk.tile([E, 3, S], F32, tag=f"mst{b}", name=f"mst{b}")
                   for b in range(Bc)]
            cstE = [wk.tile([E, 3, S], DT, tag=f"cstE{b}", name=f"cstE{b}")
                    for b in range(Bc)]
            cst = [wk.tile([S, 3 * E], DT, tag=f"cst{b}", name=f"cst{b}")
                   for b in range(Bc)]
            uS = [wk.tile([E, Bc], F32, tag=f"uS{i}", name=f"uS{i}")
                  for i in range(4)]

            with tc.tile_pool(name="psM", bufs=3, space="PSUM") as psM, \
                 tc.tile_pool(name="psC", bufs=2, space="PSUM") as psC, \
                 tc.tile_pool(name="psH", bufs=1, space="PSUM") as psH:

                def hop_stage(b, h):
                    # softmax without max-subtraction: scores are bounded
                    # (|score| < 60 for this model family; exp is f32-safe)
                    sc_ps = psH.tile([1, S], F32, space="PSUM", tag="sc")
                    nc.tensor.matmul(out=sc_ps[:],
                                     lhsT=uS[h][:, b:b + 1],
                                     rhs=mst[b][:, h, :],
                                     start=True, stop=True)
                    ex = hp.tile([1, S], F32, tag="ex")
                    zsum = hp.tile([1, 1], F32, tag="zsum")
                    nc.scalar.activation(out=ex[:], in_=sc_ps[:],
                                         func=AF.Exp, accum_out=zsum[:])
                    rz = hp.tile([1, 1], F32, tag="rz")
                    nc.vector.reciprocal(out=rz[:], in_=zsum[:])
                    exn = hp.tile([1, S], F32, tag="exn")
                    nc.vector.tensor_scalar(out=exn[:], in0=ex[:],
                                            scalar1=rz[:], scalar2=None,
                                            op0=ALU.mult)
                    pt_ps = psH.tile([S, 1], F32, space="PSUM", tag="pt")
                    nc.tensor.transpose(out=pt_ps[:], in_=exn[:],
                                        identity=id1)
                    pt = hp.tile([S, 1], DT, tag="ptb")
                    nc.vector.tensor_copy(pt[:], pt_ps[:])
                    up_ps = psH.tile([E, 1], F32, space="PSUM", tag="up")
                    nc.tensor.matmul(out=up_ps[:],
                                     lhsT=cst[b][:, h * E:(h + 1) * E],
                                     rhs=pt[:], start=True, stop=True)
                    nc.vector.tensor_tensor(out=uS[h + 1][:, b:b + 1],
                                            in0=up_ps[:],
                                            in1=uS[h][:, b:b + 1],
                                            op=ALU.add)

                def tr_stage(b):
                    # transpose c to [sent, E] for the update matmuls
                    for h in range(3):
                        cn_ps = psC.tile([S, E], DT, space="PSUM", tag="cn")
                        nc.tensor.transpose(out=cn_ps[:],
                                            in_=cstE[b][:, h, :],
                                            identity=id128)
                        nc.vector.tensor_copy(
                            cst[b][:, h * E:(h + 1) * E], cn_ps[:])

                # ---- gather chunks + reductions + per-batch hops ----------
                # Hop/transpose stages are drained one per chunk so their
                # sem waits sit between chunk matmul bursts on each
                # sequencer (no head-of-line blocking).
                import collections
                stages = collections.deque()
                L0, L1, L2, L3 = 0, 2 * S, 5 * S, 8 * S
                off16 = 0
                for (b, t0, ntiles) in c.chunks:
                    gs = ntiles * 128
                    if b < 0:
                        # query chunk
                        gq = cpool.tile([128, 1, 4 * E], DT)
                        nc.gpsimd.dma_gather(
                            out_ap=gq[:, :1, :], in_ap=t_tab.ap(),
                            idxs_ap=idxs[:, off16:off16 + gs // 16],
                            num_idxs=gs, num_idxs_reg=gs, elem_size=4 * E)
                        Pq = psM.tile([128, 9 * S], F32, space="PSUM",
                                      tag="Pm")
                        nc.tensor.matmul(out=Pq[:, 0:Bc],
                                         lhsT=gq[:, 0, 0:E], rhs=wq,
                                         start=True, stop=True)
                        nc.vector.tensor_copy(uS[0][:], Pq[:, 0:Bc])
                        off16 += gs // 16
                        continue

                    spc = ntiles * c.SPT       # sentences in this chunk
                    s0 = t0 * c.SPT            # batch-local sentence offset
                    gt = gpool.tile([128, c.gt_max, 4 * E], DT, tag="g")
                    nc.gpsimd.dma_gather(
                        out_ap=gt[:, :ntiles, :], in_ap=t_tab.ap(),
                        idxs_ap=idxs[:, off16:off16 + gs // 16],
                        num_idxs=gs, num_idxs_reg=gs, elem_size=4 * E)
                    off16 += gs // 16

                    Pm = psM.tile([128, 9 * S], F32, space="PSUM", tag="Pm")
                    for t in range(ntiles):
                        sl = 2 * t             # chunk-local sentence base
                        nc.tensor.matmul(
                            out=Pm[:, L0 + 2 * sl:L0 + 2 * sl + 4],
                            lhsT=gt[:, t, 0:E], rhs=wab,
                            start=True, stop=True)
                        nc.tensor.matmul(
                            out=Pm[:, L1 + 3 * sl:L1 + 3 * sl + 6],
                            lhsT=gt[:, t, E:2 * E], rhs=wabc,
                            start=True, stop=True)
                        nc.tensor.matmul(
                            out=Pm[:, L2 + 3 * sl:L2 + 3 * sl + 6],
                            lhsT=gt[:, t, 2 * E:3 * E], rhs=wabc,
                            start=True, stop=True)
                        nc.tensor.matmul(
                            out=Pm[:, L3 + sl:L3 + sl + 2],
                            lhsT=gt[:, t, 3 * E:4 * E], rhs=wc1,
                            start=True, stop=True)

                    # combines: m = Sa + kp*Sb + tat ; cE = Csum + tct
                    pap = Pm[:]
                    pdim = pap.ap[0]

                    def pv(off, stride):
                        return bass.AP(pap.tensor, pap.offset + off,
                                       [pdim, (stride, spc)])

                    for h, (sa, sb) in enumerate(
                            [(pv(L0, 2), pv(L0 + 1, 2)),
                             (pv(L1, 3), pv(L1 + 1, 3)),
                             (pv(L2, 3), pv(L2 + 1, 3))]):
                        msl = mst[b][:, h, s0:s0 + spc]
                        nc.vector.tensor_scalar(out=msl, in0=sb,
                                                scalar1=kp, scalar2=None,
                                                op0=ALU.mult)
                        nc.vector.tensor_tensor(out=msl, in0=msl, in1=sa,
                                                op=ALU.add)
                        nc.vector.tensor_tensor(out=msl, in0=msl,
                                                in1=tat[:, s0:s0 + spc],
                                                op=ALU.add)
                    for h, cv in enumerate([pv(L1 + 2, 3), pv(L2 + 2, 3),
                                            pv(L3, 1)]):
                        nc.vector.tensor_tensor(
                            out=cstE[b][:, h, s0:s0 + spc], in0=cv,
                            in1=tct[:, s0:s0 + spc], op=ALU.add)

                    if t0 + ntiles == c.tiles_pb:
                        stages.append(lambda bb=b: tr_stage(bb))
                        for h in range(3):
                            stages.append(lambda bb=b, hh=h:
                                          hop_stage(bb, hh))
                    if stages:
                        stages.popleft()()

                while stages:
                    stages.popleft()()

            # ---- AllGather u ----------------------------------------------
            ub_in = dram.tile([E, Bc], F32)
            ub_out = dram.tile([c.ncore * E, Bc], F32)
            nc.gpsimd.dma_start(ub_in[:], uS[3][:])
            nc.gpsimd.collective_compute(
                "AllGather", ALU.bypass,
                replica_groups=[list(range(c.ncore))],
                ins=[ub_in.opt()], outs=[ub_out.opt()],
            )
            uTf = wk.tile([E, c.ncore, Bc], F32, tag="uTf")
            src = bass.AP(ub_out[:].tensor, ub_out[:].offset,
                          [(Bc, E), (E * Bc, c.ncore), (1, Bc)])
            nc.sync.dma_start(out=uTf[:], in_=src)
            nc.sync.dma_start(out=t_du.ap(),
                              in_=uTf[:].rearrange("e c b -> e (c b)"))
            uz = wk.tile([E, c.B], c.zdt, tag="uz")
            nc.vector.tensor_copy(uz[:], uTf[:].rearrange("e c b -> e (c b)"))

            # ---- logits + log_softmax -------------------------------------
            z128 = big.tile([128, VSH // 2], c.zdt)
            sums = wk.tile([128, c.nzc], F32, tag="sums")
            with tc.tile_pool(name="psZ", bufs=4, space="PSUM") as psZ, \
                 tc.tile_pool(name="psF", bufs=1, space="PSUM") as psF:
                zo = 0
                for k, zw in enumerate(c.zws):
                    zh = zw // 2
                    zps = psZ.tile([128, 512], F32, space="PSUM", tag="z")
                    nc.tensor.matmul(out=zps[0:c.B, :zh], lhsT=uz[:],
                                     rhs=a3t[:, 2 * zo:2 * zo + zh],
                                     start=True, stop=True)
                    nc.tensor.matmul(out=zps[c.B:2 * c.B, :zh], lhsT=uz[:],
                                     rhs=a3t[:, 2 * zo + zh:2 * zo + zw],
                                     start=True, stop=True)
                    nc.vector.tensor_scalar(
                        out=z128[:, zo:zo + zh], in0=zps[:, :zh],
                        scalar1=-LOGV_SHIFT, scalar2=None, op0=ALU.add)
                    esc = hp.tile([128, 512], DT, tag="esc")
                    nc.scalar.activation(out=esc[:, :zh], in_=zps[:, :zh],
                                         func=AF.Exp,
                                         accum_out=sums[:, k:k + 1])
                    zo += zh

                slc = wk.tile([128, 1], F32, tag="slc")
                nc.vector.tensor_reduce(out=slc[:], in_=sums[:], axis=AX.X,
                                        op=ALU.add)
                sb_in = dram.tile([128, 1], F32)
                sb_out = dram.tile([128, 1], F32)
                nc.gpsimd.dma_start(sb_in[:], slc[:])
                nc.gpsimd.collective_compute(
                    "AllReduce", ALU.add,
                    replica_groups=[list(range(c.ncore))],
                    ins=[sb_in.opt()], outs=[sb_out.opt()],
                )
                st = wk.tile([128, 1], F32, tag="st")
                nc.sync.dma_start(out=st[:], in_=sb_out[:])
                # fold [128,1] -> [64,1]: total expsum per batch
                stf_ps = psF.tile([c.B, 1], F32, space="PSUM", tag="stf")
                nc.tensor.matmul(out=stf_ps[:], lhsT=wfold, rhs=st[:],
                                 start=True, stop=True)
                lseS = wk.tile([c.B, 1], F32, tag="lseS")
                nc.scalar.activation(out=lseS[:], in_=stf_ps[:], func=AF.Ln)
                # unfold [64,1] -> [128,1] and pre-shift by -ln(V)
                lse2_ps = psF.tile([128, 1], F32, space="PSUM", tag="lse2")
                nc.tensor.matmul(out=lse2_ps[:], lhsT=wunf, rhs=lseS[:],
                                 start=True, stop=True)
                lse2 = wk.tile([128, 1], F32, tag="lse2s")
                nc.vector.tensor_scalar(out=lse2[:], in0=lse2_ps[:],
                                        scalar1=-LOGV_SHIFT, scalar2=None,
                                        op0=ALU.add)
                # subtract + store output, in halves (overlap DVE with DMA)
                H = VSH // 4
                for i in range(2):
                    sl = slice(i * H, (i + 1) * H if i == 0 else VSH // 2)
                    nc.vector.tensor_scalar(out=z128[:, sl], in0=z128[:, sl],
                                            scalar1=lse2[:], scalar2=None,
                                            op0=ALU.subtract)
                    nc.sync.dma_start(out=t_o.ap()[:, sl], in_=z128[:, sl])

    nc.compile()
    return nc


def host_prep(cfg, x, q, A, TA, TC):
    c = cfg
    E, J, S = c.E, c.J, c.S
    x = np.asarray(x).astype(np.int64)
    q = np.asarray(q).astype(np.int64)
    A = np.asarray(A, dtype=np.float32)
    TA = np.asarray(TA, dtype=np.float32)
    TC = np.asarray(TC, dtype=np.float32)

    tabI = np.ascontiguousarray(A.transpose(1, 0, 2).reshape(c.V, 4 * E))
    tabI = tabI.astype(c.npdt)
    a3tF = np.ascontiguousarray(A[3].T)  # [E, V] f32

    j = np.arange(1, J + 1, dtype=np.float32)
    av = 1.0 - j / J
    bv = 2.0 * j / J - 1.0
    sp = np.arange(128) // J
    jj = np.arange(128) % J
    wab = np.zeros((128, 2 * c.SPT), np.float32)
    wabc = np.zeros((128, 3 * c.SPT), np.float32)
    wc1 = np.zeros((128, c.SPT), np.float32)
    for p in range(128):
        wab[p, 2 * sp[p] + 0] = av[jj[p]]
        wab[p, 2 * sp[p] + 1] = bv[jj[p]]
        wabc[p, 3 * sp[p] + 0] = av[jj[p]]
        wabc[p, 3 * sp[p] + 1] = bv[jj[p]]
        wabc[p, 3 * sp[p] + 2] = 1.0
        wc1[p, sp[p]] = 1.0
    wq = np.zeros((128, c.Bc), np.float32)
    for p in range(128):
        wq[p, p // c.QW] = 1.0

    tat = np.ascontiguousarray(TA[0, :S, :].T)          # [E, S] f32
    tct = np.ascontiguousarray(TC[0, :S, :].T)          # [E, S] f32
    kp = ((np.arange(E, dtype=np.float32) + 1.0) / E).reshape(E, 1)
    id1 = np.ones((1, 1), np.float32)
    id128 = np.eye(128, dtype=np.float32)
    wfold = np.zeros((128, c.B), np.float32)
    for p in range(128):
        wfold[p, p % c.B] = 1.0
    wunf = np.zeros((c.B, 128), np.float32)
    for b in range(c.B):
        wunf[b, b] = 1.0
        wunf[b, c.B + b] = 1.0

    cb = np.concatenate([wab, wabc, wc1, wq, id128], axis=1).astype(c.npdt)
    id1f = np.zeros((128, 1), np.float32)
    id1f[0, 0] = 1.0
    wunf128 = np.zeros((128, 128), np.float32)
    wunf128[:c.B, :] = wunf
    cf = np.concatenate(
        [tat, tct, kp, id1f, wfold, wunf128], axis=1).astype(np.float32)
    common = {"cb": cb, "cf": cf}

    in_maps = []
    for cc in range(c.ncore):
        qc = q[cc * c.Bc:(cc + 1) * c.Bc].reshape(-1)
        xc = x[cc * c.Bc:(cc + 1) * c.Bc].reshape(c.Bc, -1)
        xq = np.concatenate([qc] + [xc[b] for b in range(c.Bc)])
        uniq, rel = np.unique(xq, return_inverse=True)
        assert len(uniq) <= c.ucap, (len(uniq), c.ucap)
        tabc = np.zeros((c.ucap, 4 * E), c.npdt)
        tabc[:len(uniq)] = tabI[uniq]
        rel = rel.astype(np.int16)
        idx = np.zeros((128, c.TOT16), np.int16)
        off = 0
        for gs in c.gsizes:
            v = rel[off:off + gs]
            wrapped = v.reshape(-1, 16).T
            idx[:, off // 16:(off + gs) // 16] = np.tile(wrapped, (8, 1))
            off += gs
        a3c = np.ascontiguousarray(
            a3tF[:, cc * c.VSH:(cc + 1) * c.VSH]).astype(c.znp)
        m = dict(common)
        m.update({"tabc": tabc, "idx": idx, "a3t": a3c})
        in_maps.append(m)
    return in_maps


_CACHE = {}


def _get_module(cfg):
    k = cfg.key()
    if k not in _CACHE:
        _CACHE[k] = build_module(cfg)
    return _CACHE[k]


def run(cfg, inputs, trace=False):
    nc = _get_module(cfg)
    in_maps = host_prep(cfg, inputs["x"], inputs["q"], inputs["A"],
                        inputs["TA"], inputs["TC"])
    res = bass_utils.run_bass_kernel_spmd(
        nc, in_maps, core_ids=list(range(cfg.ncore)), trace=trace)
    parts = []
    for cc in range(cfg.ncore):
        z = np.asarray(res.results[cc]["o"]).astype(np.float32)
        # [128, VSH/2] -> [64, VSH]: chunk k spans zw cols; rows 0:64
        # hold the first half, rows 64:128 the second half
        zf = np.empty((cfg.B, cfg.VSH), np.float32)
        vo = zo = 0
        for zw in cfg.zws:
            zh = zw // 2
            zf[:, vo:vo + zh] = z[0:cfg.B, zo:zo + zh]
            zf[:, vo + zh:vo + zw] = z[cfg.B:2 * cfg.B, zo:zo + zh]
            vo += zw
            zo += zh
        parts.append(zf)
    out = np.concatenate(parts, axis=1)
    return out, res


def kernel(**inputs) -> np.ndarray:
    cfg = Cfg()
    out, _ = run(cfg, inputs, trace=False)
    return out
